# revision 2
# baseline (speedup 1.0000x reference)
"""Single-head attention (B=4, S=4096, E=1024, H=64) on 8 TRN2 NeuronCores.

Sharding: core c -> (batch b = c//2, sequence half h = c%2). Each core receives
only its own 2048-row x half, computes Q/K/V for it, and the core pair
(2b, 2b+1) exchanges K/V halves with chunked 2-rank AllGathers. Every core then
holds K/V for the full 4096-row sequence and computes attention for its 2048
queries.

This version is a fully software-pipelined emission schedule:
- a dummy AllGather is issued first so the collective-stream barrier (~25us,
  measured) overlaps the x load instead of serializing mid-kernel
- all 16 x-tile DMAs are enqueued up-front (16 staging buffers) so HBM input
  streams at full rate instead of being paced by the bf16 casts
- attention is emitted in AG-chunk completion order, interleaved with the
  projection chunks, and the output projection is chunked at QC=512 and woven
  between attention blocks: the PE never idles long enough for the HAM clock
  gate to re-throttle to 1.2 GHz
- the scalar (ACT) engine runs ONLY the exp activations (it is the secondary
  bottleneck at ~0.99us per [128,1024] tile); every cast/copy/mul runs on DVE
- V is PE-transposed to natural [keys, h] layout BEFORE the AllGather, so the
  readout lands directly in v_aug and phase B has no transposes

Matmuls run in bf16 (fp32 needs two PE passes); accumulation is fp32 in PSUM.
All matmuls keep K=128 with explicit zero padding (masked sub-tile matmuls
are invisible to the HAM activity monitor and leave the PE clock-gated).
The softmax denominator rides as row 64 of the context (ones column in v_aug),
is transposed by tiny PE matmuls, and its reciprocal scales the output
projection, whose row 64 of W_out carries b_out (denom * recip == 1).
"""

import sys

import numpy as np

for _p in ("/opt/trn_rl_repo",):
    if _p not in sys.path:
        sys.path.insert(0, _p)

from contextlib import ExitStack

import concourse.bass as bass  # noqa: F401  (import keeps bass registered)
import concourse.mybir as mybir
import concourse.tile as tile
from concourse import bacc, masks
from concourse.bass_utils import run_bass_kernel_spmd

F32 = mybir.dt.float32
BF16 = mybir.dt.bfloat16
AF = mybir.ActivationFunctionType
ALU = mybir.AluOpType

B, S, E, H = 4, 4096, 1024, 64
D3 = 3 * H            # 192
SH = S // 2           # queries per core
N_CORES = 8
CW = 512              # per-chunk width (rows of own half / query columns)
NCH = SH // CW        # 4 chunks
QC = 512              # query chunk for attention/output projection
ETILES = E // 128     # 8 embedding tiles
WSTR = 256            # w_sb per-e-tile stride: [K|Q|V|junk] columns
SCALE = 0.125         # 1/sqrt(H)
ST = S // 128         # 32 kj tiles over the full sequence
KEL = 64 * CW         # AG payload elems for kT [64, CW]
VEL = 128 * (CW // 2)  # AG payload elems for v natural [128, CW/2*... ] == 64*CW
WKV = KEL + VEL
REPLICA_GROUPS = [[0, 1], [2, 3], [4, 5], [6, 7]]


def _emit(nc, tc, x_ext, wq_ext, bq_ext, wo_ext, bo_ext, out_ext):
    with ExitStack() as top:
        const = top.enter_context(tc.tile_pool(name="const", bufs=1))
        dram = top.enter_context(tc.tile_pool(name="ccdram", bufs=1, space="DRAM"))

        # ---- collective-stream warmup: a dummy AllGather issued before any
        # real work triggers the one-time CC barrier while x still loads.
        warm_sb = const.tile([1, 64], BF16)
        nc.gpsimd.memset(warm_sb[:], 0.0)
        cc_w_in = dram.tile([1, 64], BF16, name="cc_w_in")
        cc_w_out = dram.tile([2, 64], BF16, name="cc_w_out")
        nc.gpsimd.dma_start(cc_w_in[0, :], warm_sb[0, :])
        nc.gpsimd.collective_compute(
            "AllGather", ALU.bypass, replica_groups=REPLICA_GROUPS,
            ins=[cc_w_in.opt()], outs=[cc_w_out.opt()],
        )

        ident = const.tile([128, 128], BF16)
        masks.make_identity(nc, ident[:])

        # ---- weight/bias staging (gpsimd DMA fp32 -> vector cast to bf16)
        wstage = top.enter_context(tc.tile_pool(name="wstage", bufs=1))
        w32 = wstage.tile([128, ETILES * D3], F32)
        nc.gpsimd.dma_start(
            w32[:].rearrange("p (e d) -> p e d", d=D3),
            wq_ext.rearrange("(e p) d -> p e d", p=128),
        )
        w_sb = const.tile([128, ETILES * WSTR], BF16)
        w_sb_v = w_sb[:].rearrange("p (e c) -> p e c", c=WSTR)
        w32_v = w32[:].rearrange("p (e c) -> p e c", c=D3)
        nc.vector.tensor_copy(w_sb_v[:, :, 0:64], w32_v[:, :, 64:128])     # K
        nc.vector.tensor_copy(w_sb_v[:, :, 64:128], w32_v[:, :, 0:64])     # Q
        nc.vector.tensor_copy(w_sb_v[:, :, 128:192], w32_v[:, :, 128:192])  # V

        bkq = const.tile([128, 1], F32)  # [b_k ; b_q]
        nc.gpsimd.dma_start(bkq[0:64, :], bq_ext[64:128].unsqueeze(1))
        nc.gpsimd.dma_start(bkq[64:128, :], bq_ext[0:64].unsqueeze(1))
        bv = const.tile([64, 1], F32)
        nc.gpsimd.dma_start(bv[:], bq_ext[128:192].unsqueeze(1))

        # W_out padded: rows 0:64 = W_out, row 64 = b_out, rows 65:128 = 0
        wo_sb = const.tile([128, E], BF16)
        nc.gpsimd.memset(wo_sb[:], 0.0)
        wo32 = wstage.tile([H, E], F32)
        nc.gpsimd.dma_start(wo32[:], wo_ext[:, :])
        nc.vector.tensor_copy(wo_sb[0:64, :], wo32[:])
        bo32 = wstage.tile([1, E], F32)
        nc.gpsimd.dma_start(bo32[:], bo_ext.unsqueeze(0))
        bo16 = wstage.tile([1, E], BF16)
        nc.vector.tensor_copy(bo16[:], bo32[:])
        nc.gpsimd.dma_start(wo_sb[64:65, :], bo16[:])

        # ---- persistent attention operands (global kv order on free axis)
        kt_sb = const.tile([128, S], BF16)   # kT on rows 0:64, zeros below
        nc.gpsimd.memset(kt_sb[64:128, :], 0.0)
        v_aug = const.tile([128, ST * 128], BF16)  # [keys, 64 v | 1 | zeros]
        nc.gpsimd.memset(v_aug[:], 0.0)
        nc.gpsimd.memset(
            v_aug[:].rearrange("p (t c) -> p t c", c=128)[:, :, 64:65], 1.0
        )
        q2_sb = const.tile([128, SH], BF16)  # qT on rows 0:64, zeros below
        nc.gpsimd.memset(q2_sb[:], 0.0)
        ones11 = const.tile([1, 1], BF16)
        nc.gpsimd.memset(ones11[:], 1.0)

        # ---- pools
        xsb = top.enter_context(tc.tile_pool(name="xsb", bufs=16))
        xbp = top.enter_context(tc.tile_pool(name="xbp", bufs=6))
        xTp = top.enter_context(tc.tile_pool(name="xTp", bufs=2))
        kqp = top.enter_context(tc.tile_pool(name="kqp", bufs=4))
        vstp = top.enter_context(tc.tile_pool(name="vstp", bufs=2))
        vsp = top.enter_context(tc.tile_pool(name="vsp", bufs=2))
        expp = top.enter_context(tc.tile_pool(name="expp", bufs=6))
        cbp = top.enter_context(tc.tile_pool(name="cbp", bufs=2))
        rsp = top.enter_context(tc.tile_pool(name="rsp", bufs=2))
        outp = top.enter_context(tc.tile_pool(name="outp", bufs=3))
        # PSUM: xtp(2) + m1p(1) + m2p(1) + sps(2) + cps(2) = 8 banks
        xtp = top.enter_context(tc.tile_pool(name="xtp", bufs=2, space="PSUM"))
        m1p = top.enter_context(tc.tile_pool(name="m1p", bufs=1, space="PSUM"))
        m2p = top.enter_context(tc.tile_pool(name="m2p", bufs=1, space="PSUM"))
        sps = top.enter_context(tc.tile_pool(name="sps", bufs=2, space="PSUM"))
        cps = top.enter_context(tc.tile_pool(name="cps", bufs=2, space="PSUM"))

        cc_in = [dram.tile([1, WKV], BF16, name=f"cc_in{c}") for c in range(NCH)]
        cc_out = [dram.tile([2, WKV], BF16, name=f"cc_out{c}") for c in range(NCH)]

        # ---- all x DMAs up-front on both HWDGE queues: with 16 staging
        # buffers nothing paces them, so input streams at full HBM rate
        t32s = []
        for k in range(4 * NCH):
            t32 = xsb.tile([128, E], F32, name="t32", tag="t32")
            eng = nc.sync if k % 2 == 0 else nc.scalar
            eng.dma_start(t32[:], x_ext[k * 128:(k + 1) * 128, :])
            t32s.append(t32)

        xbs = {}

        def emit_casts(c):
            for t in range(4):
                k = 4 * c + t
                xb = xbp.tile([128, E], BF16, name="xb", tag="xb")
                nc.vector.tensor_copy(xb[:], t32s[k][:])
                xbs[k] = xb

        kqs = []
        v_aug_v = v_aug[:].rearrange("p (j c) -> p j c", c=128)

        def emit_tp(c):
            """transpose + project + v-transpose + AG stage/issue for chunk c"""
            xT = xTp.tile([128, ETILES * CW], BF16, name="xT", tag="xT")
            for e in range(ETILES):
                p = xtp.tile([128, CW], F32, name="xtps", tag="xtps")
                for t in range(4):
                    nc.tensor.matmul(
                        p[:, t * 128:(t + 1) * 128],
                        xbs[4 * c + t][:, e * 128:(e + 1) * 128],
                        ident[:],
                    )
                nc.vector.tensor_copy(xT[:, e * CW:(e + 1) * CW], p[:])
            m1 = m1p.tile([128, CW], F32, name="m1")
            m2 = m2p.tile([128, CW], F32, name="m2")
            for e in range(ETILES):
                rhs = xT[:, e * CW:(e + 1) * CW]
                nc.tensor.matmul(
                    m1[:], w_sb[:, e * WSTR:e * WSTR + 128], rhs,
                    start=(e == 0), stop=(e == ETILES - 1),
                )
                nc.tensor.matmul(
                    m2[:], w_sb[:, e * WSTR + 128:e * WSTR + 256], rhs,
                    start=(e == 0), stop=(e == ETILES - 1),
                )
            kq = kqp.tile([128, CW], BF16, name="kq", tag="kq")
            nc.vector.tensor_scalar_add(kq[:], m1[:], bkq[:])
            kqs.append(kq)
            vst = vstp.tile([64, CW], BF16, name="vst", tag="vst")
            nc.vector.tensor_scalar_add(vst[:], m2[0:64, :], bv[:])
            # v to natural [keys, h] layout before the AllGather
            vp = xtp.tile([128, CW], F32, name="xtps", tag="xtps")
            for t in range(4):
                nc.tensor.matmul(
                    vp[:, t * 64:(t + 1) * 64],
                    vst[:, t * 128:(t + 1) * 128],
                    ident[0:64, 0:64],
                )
            v_nat = vsp.tile([128, CW // 2], BF16, name="v_nat", tag="v_nat")
            nc.vector.tensor_copy(v_nat[:], vp[:, 0:CW // 2])
            # stage + AllGather (gpsimd stream)
            nc.gpsimd.dma_start(cc_in[c][0, 0:KEL], kq[0:64, :])
            nc.gpsimd.dma_start(cc_in[c][0, KEL:WKV], v_nat[:])
            nc.gpsimd.collective_compute(
                "AllGather", ALU.bypass, replica_groups=REPLICA_GROUPS,
                ins=[cc_in[c].opt()], outs=[cc_out[c].opt()],
            )
            # q columns into the padded moving operand (sync HWDGE queue so
            # the gpsimd CC stream never waits on it)
            nc.sync.dma_start(
                q2_sb[0:64, c * CW:(c + 1) * CW], kq[64:128, :]
            )

        def emit_readout(g):
            for r in range(2):
                cols = slice(r * SH + g * CW, r * SH + (g + 1) * CW)
                nc.gpsimd.dma_start(
                    kt_sb[0:64, cols],
                    cc_out[g][r, 0:KEL].rearrange("(p f) -> p f", p=64),
                )
                j0 = r * 16 + g * 4
                nc.gpsimd.dma_start(
                    v_aug_v[:, j0:j0 + 4, 0:64],
                    cc_out[g][r, KEL:WKV].rearrange(
                        "(p t c) -> p t c", p=128, t=4
                    ),
                )

        ctxs = {}
        kj_count = {}

        def emit_attn(x, g):
            """attention block: query chunk x against AG-chunk g (8 kj tiles)"""
            if x not in ctxs:
                ctxs[x] = cps.tile([128, QC], F32, name="ctx", tag="ctx")
                kj_count[x] = 0
            ctx = ctxs[x]
            q0 = x * QC
            for r in range(2):
                for t in range(4):
                    j = r * 16 + g * 4 + t
                    i = kj_count[x]
                    kj_count[x] += 1
                    sc = sps.tile([128, QC], F32, name="sc", tag="sc")
                    nc.tensor.matmul(
                        sc[:],
                        kt_sb[:, j * 128:(j + 1) * 128],
                        q2_sb[:, q0:q0 + QC],
                    )
                    ex = expp.tile([128, QC], BF16, name="ex", tag="ex")
                    nc.scalar.activation(ex[:], sc[:], AF.Exp, scale=SCALE)
                    nc.tensor.matmul(
                        ctx[:],
                        v_aug[:, j * 128:(j + 1) * 128],
                        ex[:],
                        start=(i == 0), stop=(i == ST - 1),
                        skip_group_check=True,
                    )

        out_q = [nc.sync, nc.scalar]

        def emit_phc(x):
            """output projection for query chunk x"""
            ctx = ctxs.pop(x)
            q0 = x * QC
            # rows 65:128 of ctx are exact zeros (v_aug zero padding), so the
            # bf16 copy takes all 128 rows and the out matmul K=128 is safe
            ctx16 = cbp.tile([128, QC], BF16, name="ctx16", tag="ctx16")
            nc.vector.tensor_copy(ctx16[:], ctx[:])
            rs_row = rsp.tile([1, QC], BF16, name="rs_row", tag="rs_row")
            nc.vector.tensor_copy(rs_row[:], ctx16[64:65, :])
            rs_ps = xtp.tile([128, CW], F32, name="xtps", tag="xtps")
            for i in range(QC // 128):
                nc.tensor.matmul(
                    rs_ps[:, i:i + 1],
                    rs_row[0:1, i * 128:(i + 1) * 128],
                    ones11[:],
                )
            recip = rsp.tile([128, QC // 128], F32, name="recip", tag="recip")
            nc.vector.reciprocal(recip[:], rs_ps[:, 0:QC // 128])
            for i in range(QC // 128):
                out_sb = outp.tile([128, E], F32, name="out_sb", tag="out_sb")
                for n in range(2):
                    op = xtp.tile([128, CW], F32, name="xtps", tag="xtps")
                    nc.tensor.matmul(
                        op[:],
                        ctx16[:, i * 128:(i + 1) * 128],
                        wo_sb[:, n * 512:(n + 1) * 512],
                    )
                    nc.vector.tensor_scalar_mul(
                        out_sb[:, n * 512:(n + 1) * 512], op[:],
                        recip[:, i:i + 1],
                    )
                out_q[i % 2].dma_start(
                    out_ext[q0 + i * 128:q0 + (i + 1) * 128, :], out_sb[:]
                )

        # ---- the schedule ------------------------------------------------
        emit_casts(0)
        emit_casts(1)
        emit_tp(0)
        emit_casts(2)
        emit_tp(1)
        emit_readout(0)           # gpsimd: after AG0/AG1 issues
        emit_casts(3)
        emit_tp(2)
        emit_tp(3)
        emit_readout(1)
        emit_readout(2)
        emit_readout(3)

        for x, g in [(0, 0), (0, 1), (1, 0), (1, 1), (0, 2), (1, 2),
                     (0, 3), (1, 3)]:
            emit_attn(x, g)
        emit_phc(0)
        emit_attn(2, 0)
        emit_attn(2, 1)
        emit_phc(1)
        emit_attn(2, 2)
        emit_attn(2, 3)
        emit_attn(3, 0)
        emit_attn(3, 1)
        emit_phc(2)
        emit_attn(3, 2)
        emit_attn(3, 3)
        emit_phc(3)


_NC = None


def _get_nc():
    global _NC
    if _NC is None:
        nc = bacc.Bacc("TRN2", target_bir_lowering=False, debug=False,
                       num_devices=N_CORES)
        x_ext = nc.dram_tensor("x", [SH, E], F32, kind="ExternalInput").ap()
        wq_ext = nc.dram_tensor("w_qkv", [E, D3], F32, kind="ExternalInput").ap()
        bq_ext = nc.dram_tensor("b_qkv", [D3], F32, kind="ExternalInput").ap()
        wo_ext = nc.dram_tensor("w_out", [H, E], F32, kind="ExternalInput").ap()
        bo_ext = nc.dram_tensor("b_out", [E], F32, kind="ExternalInput").ap()
        out_ext = nc.dram_tensor("out", [SH, E], F32, kind="ExternalOutput").ap()
        with tile.TileContext(nc) as tc:
            _emit(nc, tc, x_ext, wq_ext, bq_ext, wo_ext, bo_ext, out_ext)
        nc.compile()
        _NC = nc
    return _NC


last_results = None
last_tmpdir = None


def kernel(x, W_qkv, b_qkv, W_out, b_out):
    nc = _get_nc()
    x = np.ascontiguousarray(x, dtype=np.float32)
    shared = {
        "w_qkv": np.ascontiguousarray(W_qkv, dtype=np.float32),
        "b_qkv": np.ascontiguousarray(b_qkv, dtype=np.float32),
        "w_out": np.ascontiguousarray(W_out, dtype=np.float32),
        "b_out": np.ascontiguousarray(b_out, dtype=np.float32),
    }
    in_maps = []
    for c in range(N_CORES):
        b, h = divmod(c, 2)
        xp = np.ascontiguousarray(x[b, h * SH:(h + 1) * SH])
        in_maps.append({"x": xp, **shared})

    import os
    import tempfile
    import time

    tmpdir = os.environ.get("ATTN_TRACE_DIR") or tempfile.mkdtemp(prefix="attn_trace_")
    res = None
    for attempt in range(3):
        try:
            res = run_bass_kernel_spmd(
                nc, in_maps, core_ids=list(range(N_CORES)), tmpdir=tmpdir
            )
            break
        except Exception:
            # transient NRT_EXEC_UNIT_UNRECOVERABLE has been observed on a
            # first attempt; a clean retry recovers
            if attempt == 2:
                raise
            time.sleep(2.0)
    global last_results, last_tmpdir
    last_results = res
    last_tmpdir = tmpdir

    out = np.empty((B, S, E), dtype=np.float32)
    for c in range(N_CORES):
        b, h = divmod(c, 2)
        out[b, h * SH:(h + 1) * SH] = res.results[c]["out"]
    return out
